# revision 17
# baseline (speedup 1.0000x reference)
"""Trainium2 Bass kernel for nn_CrossAttention_79448305041860.

Dual cross-attention (q1, q2 vs shared kv) + concat + out-proj + LayerNorm,
B=4, E=256, N=64*64=4096 tokens.

Sharding: 8 cores = 4 batches x 2 query-token halves. Each core computes
K,V for its batch (replicated across the pair of cores sharing a batch) and
the full pipeline for its 2048-query-token slice. No cross-core comm.

Per-core layout strategy:
  - K^T, Q^T computed E-major [e, tokens] straight from the channel-major
    inputs (contraction c on partitions) -- no input transposes.
  - Scores computed transposed: S^T[k, q] = (K^T-slice).T @ Q^T. exp() on
    the scalar engine (scale=1/16 folded in; scores are O(1) so no
    max-subtraction is needed for fp32 stability).
  - P^T tiles feed the PV matmul directly as the moving operand:
    out^T[e, q] += V[k-tile, e].T @ P^T[k-tile, q]  (V is token-major).
  - Softmax denominators: vector-engine accumulation of P^T tiles over k,
    then a tiny N=1 matmul with a ones-vector per 128 queries gives
    denom[q, 1] with q on partitions.
  - Out-proj consumes the *unnormalized* out^T halves as stationary
    operands: y_half[n-tile, e] = outT[f, n-tile].T @ woT[f, e]; the
    1/denom factors become native per-partition scalars there.
  - LayerNorm along the free dim (bn_stats/bn_aggr), then 128x128 PE
    transposes to the channel-major output layout.

All matmuls run in float32r (FP22 single-pass, full PE rate); walrus
requires fp32r matmul inputs to be produced as fp32r, so every matmul
input tile is allocated with that dtype.
"""

import numpy as np
from contextlib import ExitStack

import concourse.bass as bass
import concourse.mybir as mybir
import concourse.tile as tile
from concourse import bacc
from concourse.masks import make_identity

FP32 = mybir.dt.float32
FP32R = mybir.dt.float32r
AF = mybir.ActivationFunctionType
ALU = mybir.AluOpType

P = 128
B = 4
E = 256            # embed dim
ET = E // P        # 2 e-tiles
CKV = 512          # kv channels
CT = CKV // P      # 4 c-tiles
CQ = 256           # q channels
CQT = CQ // P      # 2 c-tiles
N = 4096           # kv tokens per batch
NKT = N // P       # 32 k token-tiles
NQ = 2048          # query tokens per core
QB = 512           # q block (psum bank width)
NQB = NQ // QB     # 4 q blocks
NT = NQ // P       # 16 token-tiles per core
SCALE = 1.0 / 16.0  # 1/sqrt(E)
LN_EPS = 1e-5


def _bcast_row(nc, dram_handle, sbuf_tile):
    """DMA-broadcast a [E] dram vector to all partitions of a [P, E] tile."""
    src_ap = dram_handle[:]
    bcast = bass.AP(
        tensor=src_ap.tensor,
        offset=src_ap.offset,
        ap=[[0, P], *src_ap.ap],
    )
    nc.gpsimd.dma_start(out=sbuf_tile[:], in_=bcast)


def build_nc():
    nc = bacc.Bacc()

    xq1_d = nc.dram_tensor("xq1", [CQ, NQ], FP32, kind="ExternalInput")
    xq2_d = nc.dram_tensor("xq2", [CQ, NQ], FP32, kind="ExternalInput")
    xkv_d = nc.dram_tensor("xkv", [CKV, N], FP32, kind="ExternalInput")
    wq1t_d = nc.dram_tensor("wq1t", [CQ, E], FP32, kind="ExternalInput")
    wq2t_d = nc.dram_tensor("wq2t", [CQ, E], FP32, kind="ExternalInput")
    wkt_d = nc.dram_tensor("wkt", [CKV, E], FP32, kind="ExternalInput")
    wvt_d = nc.dram_tensor("wvt", [CKV, E], FP32, kind="ExternalInput")
    wo1t_d = nc.dram_tensor("wo1t", [E, E], FP32, kind="ExternalInput")
    wo2t_d = nc.dram_tensor("wo2t", [E, E], FP32, kind="ExternalInput")
    bq1_d = nc.dram_tensor("bq1", [E], FP32, kind="ExternalInput")
    bq2_d = nc.dram_tensor("bq2", [E], FP32, kind="ExternalInput")
    bk_d = nc.dram_tensor("bk", [E], FP32, kind="ExternalInput")
    bv_d = nc.dram_tensor("bv", [E], FP32, kind="ExternalInput")
    bo_d = nc.dram_tensor("bo", [E], FP32, kind="ExternalInput")
    lnw_d = nc.dram_tensor("lnw", [E], FP32, kind="ExternalInput")
    lnb_d = nc.dram_tensor("lnb", [E], FP32, kind="ExternalInput")
    out_d = nc.dram_tensor("out", [E, NQ], FP32, kind="ExternalOutput")

    with tile.TileContext(nc) as tc, ExitStack() as ctx:
        const = ctx.enter_context(tc.tile_pool(name="const", bufs=1))
        wts = ctx.enter_context(tc.tile_pool(name="wts", bufs=1))
        bigin = ctx.enter_context(tc.tile_pool(name="bigin", bufs=1))
        keep = ctx.enter_context(tc.tile_pool(name="keep", bufs=1))
        flow = ctx.enter_context(tc.tile_pool(name="flow", bufs=1))
        ps_s = ctx.enter_context(tc.tile_pool(name="ps_s", bufs=2, space="PSUM"))
        ps_o = ctx.enter_context(tc.tile_pool(name="ps_o", bufs=4, space="PSUM"))
        ps_d = ctx.enter_context(tc.tile_pool(name="ps_d", bufs=2, space="PSUM"))

        # ---- constants ----
        ident = const.tile([P, P], FP32, name="ident")
        make_identity(nc, ident)
        # fp32r matmuls need an even moving-operand innermost count, so the
        # denominator reduction uses a [P, 2] ones operand (N=2).
        ones_raw = const.tile([P, 2], FP32, name="ones_raw")
        nc.vector.memset(ones_raw, 1.0)
        ones = const.tile([P, 2], FP32R, name="ones")
        nc.vector.tensor_copy(ones[:], ones_raw[:])
        epst = const.tile([P, 1], FP32, name="epst")
        nc.vector.memset(epst, LN_EPS)

        # ---- weights / biases ----
        def _load_w(name, dram, ctiles):
            t = wts.tile([P, ctiles, E], FP32R, name=name)
            nc.sync.dma_start(
                t[:], dram[:].bitcast(FP32R).rearrange("(o p) e -> p o e", p=P)
            )
            return t

        wq1t = _load_w("wq1t", wq1t_d, CQT)
        wq2t = _load_w("wq2t", wq2t_d, CQT)
        wkt = _load_w("wkt", wkt_d, CT)
        wvt = _load_w("wvt", wvt_d, CT)
        wo1t = _load_w("wo1t", wo1t_d, ET)
        wo2t = _load_w("wo2t", wo2t_d, ET)

        bq1 = wts.tile([P, ET], FP32, name="bq1")
        nc.sync.dma_start(bq1[:], bq1_d[:].rearrange("(o p) -> p o", p=P))
        bq2 = wts.tile([P, ET], FP32, name="bq2")
        nc.sync.dma_start(bq2[:], bq2_d[:].rearrange("(o p) -> p o", p=P))
        bk = wts.tile([P, ET], FP32, name="bk")
        nc.sync.dma_start(bk[:], bk_d[:].rearrange("(o p) -> p o", p=P))

        bv_b = wts.tile([P, E], FP32, name="bv_b")
        _bcast_row(nc, bv_d, bv_b)
        bo_b = wts.tile([P, E], FP32, name="bo_b")
        _bcast_row(nc, bo_d, bo_b)
        lnw_b = wts.tile([P, E], FP32, name="lnw_b")
        _bcast_row(nc, lnw_d, lnw_b)
        lnb_b = wts.tile([P, E], FP32, name="lnb_b")
        _bcast_row(nc, lnb_d, lnb_b)

        # ---- phase 0: K^T, V, Q^T projections ----
        ktm = keep.tile([P, ET, N], FP32R, name="ktm")    # K^T e-major
        vtm = keep.tile([P, NKT, E], FP32R, name="vtm")   # V token-major

        KVCH = 512  # kv token-columns per streamed chunk
        for ch in range(N // KVCH):
            xkv_sb = bigin.tile([P, CT, KVCH], FP32R, name="xkv", tag="xkv", bufs=2)
            nc.sync.dma_start(
                xkv_sb[:],
                xkv_d[:].bitcast(FP32R).rearrange("(o p) n -> p o n", p=P)[
                    :, :, ch * KVCH : (ch + 1) * KVCH
                ],
            )
            # K^T for these token-columns
            for t in range(ET):
                for cc in range(KVCH // QB):
                    ps = ps_s.tile([P, QB], FP32, name="kps", tag="s")
                    for j in range(CT):
                        nc.tensor.matmul(
                            ps[:],
                            wkt[:, j, t * P : (t + 1) * P],
                            xkv_sb[:, j, cc * QB : (cc + 1) * QB],
                            start=(j == 0),
                            stop=(j == CT - 1),
                        )
                    nc.scalar.activation(
                        ktm[:, t, ch * KVCH + cc * QB : ch * KVCH + (cc + 1) * QB],
                        ps[:],
                        AF.Identity,
                        bias=bk[:, t : t + 1],
                        scale=1.0,
                    )
            # V for these token-rows
            for v in range(KVCH // P):
                kt_idx = ch * (KVCH // P) + v
                ps = ps_o.tile([P, E], FP32, name="vps", tag="o")
                for j in range(CT):
                    nc.tensor.matmul(
                        ps[:],
                        xkv_sb[:, j, v * P : (v + 1) * P],
                        wvt[:, j, :],
                        start=(j == 0),
                        stop=(j == CT - 1),
                    )
                nc.vector.tensor_tensor(vtm[:, kt_idx, :], ps[:], bv_b[:], ALU.add)

        qt1 = keep.tile([P, CQT, NQ], FP32R, name="qt1")  # Q1^T e-major
        qt2 = keep.tile([P, CQT, NQ], FP32R, name="qt2")
        QCH = 1024  # q token-columns per streamed chunk (double-buffered)
        for xq_d, wqt, bq, qt in ((xq1_d, wq1t, bq1, qt1), (xq2_d, wq2t, bq2, qt2)):
            for ch in range(NQ // QCH):
                csl = slice(ch * QCH, (ch + 1) * QCH)
                xq_sb = bigin.tile([P, CQT, QCH], FP32R, name="xq", tag="xq", bufs=2)
                nc.sync.dma_start(
                    xq_sb[:],
                    xq_d[:].bitcast(FP32R).rearrange("(o p) n -> p o n", p=P)[
                        :, :, csl
                    ],
                )
                for t in range(ET):
                    for cc in range(QCH // QB):
                        ps = ps_s.tile([P, QB], FP32, name="qps", tag="s")
                        for j in range(CQT):
                            nc.tensor.matmul(
                                ps[:],
                                wqt[:, j, t * P : (t + 1) * P],
                                xq_sb[:, j, cc * QB : (cc + 1) * QB],
                                start=(j == 0),
                                stop=(j == CQT - 1),
                            )
                        nc.scalar.activation(
                            qt[:, t, ch * QCH + cc * QB : ch * QCH + (cc + 1) * QB],
                            ps[:],
                            AF.Identity,
                            bias=bq[:, t : t + 1],
                            scale=1.0,
                        )

        # ---- attention + out-proj, interleaved per q-block ----
        # Each q-block runs both attentions, then immediately the out-proj /
        # LayerNorm / transpose for its tokens -- the Tile scheduler overlaps
        # the (DVE/ACT-heavy) epilogue of block b with the (PE-heavy)
        # attention of block b+1.
        o1ut = keep.tile([P, ET, NQ], FP32R, name="o1ut")  # unnormalized out1^T
        o2ut = keep.tile([P, ET, NQ], FP32R, name="o2ut")
        r1 = keep.tile([P, NT], FP32, name="r1")           # 1/denom per token
        r2 = keep.tile([P, NT], FP32, name="r2")
        out_r = out_d[:].rearrange("(o p) n -> p o n", p=P)

        for qb in range(NQB):
            qsl = slice(qb * QB, (qb + 1) * QB)
            for qt, out_t, r_t in ((qt1, o1ut, r1), (qt2, o2ut, r2)):
                o_ps = [
                    ps_o.tile([P, QB], FP32, name=f"ops{t}", tag="o")
                    for t in range(ET)
                ]
                acc = flow.tile([P, QB], FP32R, name="acc", tag="acc", bufs=2)
                for k in range(NKT):
                    s_ps = ps_s.tile([P, QB], FP32, name="sps", tag="s")
                    for t in range(ET):
                        nc.tensor.matmul(
                            s_ps[:],
                            ktm[:, t, k * P : (k + 1) * P],
                            qt[:, t, qsl],
                            start=(t == 0),
                            stop=(t == ET - 1),
                        )
                    pt = flow.tile([P, QB], FP32R, name="pt", tag="pt", bufs=3)
                    nc.scalar.activation(pt[:], s_ps[:], AF.Exp, scale=SCALE)
                    for t in range(ET):
                        nc.tensor.matmul(
                            o_ps[t][:],
                            vtm[:, k, t * P : (t + 1) * P],
                            pt[:],
                            start=(k == 0),
                            stop=(k == NKT - 1),
                        )
                    if k == 0:
                        nc.vector.tensor_copy(acc[:], pt[:])
                    else:
                        nc.vector.tensor_tensor(acc[:], acc[:], pt[:], ALU.add)
                for t in range(ET):
                    nc.vector.tensor_copy(out_t[:, t, qsl], o_ps[t][:])
                # denominators -> [q, 1] via ones-vector matmuls (N=2 for
                # the fp32r even-element restriction; column 0 is used)
                d_ps = ps_d.tile([P, QB // P, 2], FP32, name="dps", tag="d")
                for i in range(QB // P):
                    nc.tensor.matmul(
                        d_ps[:, i, :],
                        acc[:, i * P : (i + 1) * P],
                        ones[:],
                        start=True,
                        stop=True,
                    )
                nc.vector.reciprocal(
                    r_t[:, qb * (QB // P) : (qb + 1) * (QB // P)], d_ps[:, :, 0]
                )

            # out-proj + LN + transpose for this q-block's tokens
            for nt in range(qb * (QB // P), (qb + 1) * (QB // P)):
                nsl = slice(nt * P, (nt + 1) * P)
                y1_ps = ps_d.tile([P, E], FP32, name="y1ps", tag="d")
                for j in range(ET):
                    nc.tensor.matmul(
                        y1_ps[:],
                        o1ut[:, j, nsl],
                        wo1t[:, j, :],
                        start=(j == 0),
                        stop=(j == ET - 1),
                    )
                y2_ps = ps_d.tile([P, E], FP32, name="y2ps", tag="d")
                for j in range(ET):
                    nc.tensor.matmul(
                        y2_ps[:],
                        o2ut[:, j, nsl],
                        wo2t[:, j, :],
                        start=(j == 0),
                        stop=(j == ET - 1),
                    )
                y = flow.tile([P, E], FP32, name="y", tag="y", bufs=3)
                y2 = flow.tile([P, E], FP32, name="y2", tag="y2", bufs=3)
                nc.vector.tensor_scalar_mul(y[:], y1_ps[:], r1[:, nt : nt + 1])
                nc.vector.tensor_scalar_mul(y2[:], y2_ps[:], r2[:, nt : nt + 1])
                nc.vector.tensor_add(y[:], y[:], y2[:])
                nc.vector.tensor_add(y[:], y[:], bo_b[:])
                # LayerNorm over the free dim
                st6 = flow.tile([P, 6], FP32, name="st6", tag="st6", bufs=3)
                mv = flow.tile([P, 2], FP32, name="mv", tag="mv", bufs=3)
                nc.vector.bn_stats(out=st6[:], in_=y[:])
                nc.vector.bn_aggr(out=mv[:], in_=st6[:])
                rstd = flow.tile([P, 1], FP32, name="rstd", tag="rstd", bufs=3)
                nc.scalar.activation(
                    rstd[:], mv[:, 1:2], AF.Sqrt, bias=epst[:], scale=1.0
                )
                nc.vector.reciprocal(rstd[:], rstd[:])
                nc.vector.tensor_scalar(
                    y[:], y[:], mv[:, 0:1], rstd[:], op0=ALU.subtract, op1=ALU.mult
                )
                nc.vector.tensor_tensor(y[:], y[:], lnw_b[:], ALU.mult)
                nc.vector.tensor_tensor(y[:], y[:], lnb_b[:], ALU.add)
                yt = flow.tile([P, ET, P], FP32, name="yt", tag="yt", bufs=3)
                for t in range(ET):
                    tp = ps_s.tile([P, P], FP32, name="tp", tag="s")
                    nc.tensor.transpose(tp[:], y[:, t * P : (t + 1) * P], ident[:])
                    nc.vector.tensor_copy(yt[:, t, :], tp[:])
                for t in range(ET):
                    nc.sync.dma_start(out_r[:, t, nsl], yt[:, t, :])

    nc.compile()
    return nc


_CACHE = {}


def _get_nc():
    if "nc" not in _CACHE:
        _CACHE["nc"] = build_nc()
    return _CACHE["nc"]


def make_in_maps(q1, q2, kv, wq1, bq1, wq2, bq2, wk, bk, wv, bv, wo, bo, ln_w, ln_b):
    f32 = lambda a: np.ascontiguousarray(np.asarray(a, dtype=np.float32))
    q1, q2, kv = f32(q1), f32(q2), f32(kv)
    base = {
        "wq1t": f32(np.asarray(wq1).T),
        "wq2t": f32(np.asarray(wq2).T),
        "wkt": f32(np.asarray(wk).T),
        "wvt": f32(np.asarray(wv).T),
        "wo1t": f32(np.asarray(wo)[:, :E].T),
        "wo2t": f32(np.asarray(wo)[:, E:].T),
        "bq1": f32(bq1),
        "bq2": f32(bq2),
        "bk": f32(bk),
        "bv": f32(bv),
        "bo": f32(bo),
        "lnw": f32(ln_w),
        "lnb": f32(ln_b),
    }
    kv_flat = [f32(kv[b].reshape(CKV, N)) for b in range(B)]
    in_maps = []
    for c in range(8):
        b, h = divmod(c, 2)
        m = dict(base)
        m["xq1"] = f32(q1[b, :, h * 32 : (h + 1) * 32, :].reshape(CQ, NQ))
        m["xq2"] = f32(q2[b, :, h * 32 : (h + 1) * 32, :].reshape(CQ, NQ))
        m["xkv"] = kv_flat[b]
        in_maps.append(m)
    return in_maps


def assemble_output(results):
    out = np.empty((B, E, 64, 64), dtype=np.float32)
    for c in range(8):
        b, h = divmod(c, 2)
        out[b, :, h * 32 : (h + 1) * 32, :] = results[c]["out"].reshape(E, 32, 64)
    return out


def kernel(**inputs):
    from concourse.bass_utils import run_bass_kernel_spmd

    nc = _get_nc()
    in_maps = make_in_maps(**inputs)
    res = run_bass_kernel_spmd(nc, in_maps, list(range(8)))
    return assemble_output(res.results)


if __name__ == "__main__":
    nc = build_nc()
    print("built ok")


# revision 19
# speedup vs baseline: 1.2115x; 1.2115x over previous
"""Trainium2 Bass kernel for nn_CrossAttention_79448305041860.

Dual cross-attention (q1, q2 vs shared kv) + concat + out-proj + LayerNorm,
B=4, E=256, N=64*64=4096 tokens.

Sharding: 8 cores = 4 batches x 2 query-token halves. Each core computes
K,V for its batch (replicated across the pair of cores sharing a batch) and
the full pipeline for its 2048-query-token slice. No cross-core comm.

Per-core layout strategy:
  - K^T, Q^T computed E-major [e, tokens] straight from the channel-major
    inputs (contraction c on partitions) -- no input transposes.
  - Scores computed transposed: S^T[k, q] = (K^T-slice).T @ Q^T. exp() on
    the scalar engine (scale=1/16 folded in; scores are O(1) so no
    max-subtraction is needed for fp32 stability).
  - P^T tiles feed the PV matmul directly as the moving operand:
    out^T[e, q] += V[k-tile, e].T @ P^T[k-tile, q]  (V is token-major).
  - Softmax denominators: vector-engine accumulation of P^T tiles over k,
    then a tiny N=1 matmul with a ones-vector per 128 queries gives
    denom[q, 1] with q on partitions.
  - Out-proj consumes the *unnormalized* out^T halves as stationary
    operands: y_half[n-tile, e] = outT[f, n-tile].T @ woT[f, e]; the
    1/denom factors become native per-partition scalars there.
  - LayerNorm along the free dim (bn_stats/bn_aggr), then 128x128 PE
    transposes to the channel-major output layout.

All matmuls run in float32r (FP22 single-pass, full PE rate); walrus
requires fp32r matmul inputs to be produced as fp32r, so every matmul
input tile is allocated with that dtype.
"""

import numpy as np
from contextlib import ExitStack

import concourse.bass as bass
import concourse.mybir as mybir
import concourse.tile as tile
from concourse import bacc
from concourse.masks import make_identity

FP32 = mybir.dt.float32
FP32R = mybir.dt.float32r
AF = mybir.ActivationFunctionType
ALU = mybir.AluOpType

P = 128
B = 4
E = 256            # embed dim
ET = E // P        # 2 e-tiles
CKV = 512          # kv channels
CT = CKV // P      # 4 c-tiles
CQ = 256           # q channels
CQT = CQ // P      # 2 c-tiles
N = 4096           # kv tokens per batch
NKT = N // P       # 32 k token-tiles
NQ = 2048          # query tokens per core
QB = 512           # q block (psum bank width)
NQB = NQ // QB     # 4 q blocks
NT = NQ // P       # 16 token-tiles per core
SCALE = 1.0 / 16.0  # 1/sqrt(E)
LN_EPS = 1e-5


def _bcast_row(nc, dram_handle, sbuf_tile):
    """DMA-broadcast a [E] dram vector to all partitions of a [P, E] tile."""
    src_ap = dram_handle[:]
    bcast = bass.AP(
        tensor=src_ap.tensor,
        offset=src_ap.offset,
        ap=[[0, P], *src_ap.ap],
    )
    nc.gpsimd.dma_start(out=sbuf_tile[:], in_=bcast)


def build_nc():
    nc = bacc.Bacc()

    xq1_d = nc.dram_tensor("xq1", [CQ, NQ], FP32, kind="ExternalInput")
    xq2_d = nc.dram_tensor("xq2", [CQ, NQ], FP32, kind="ExternalInput")
    xkv_d = nc.dram_tensor("xkv", [CKV, N], FP32, kind="ExternalInput")
    wq1t_d = nc.dram_tensor("wq1t", [CQ, E], FP32, kind="ExternalInput")
    wq2t_d = nc.dram_tensor("wq2t", [CQ, E], FP32, kind="ExternalInput")
    wkt_d = nc.dram_tensor("wkt", [CKV, E], FP32, kind="ExternalInput")
    wvt_d = nc.dram_tensor("wvt", [CKV, E], FP32, kind="ExternalInput")
    wo1t_d = nc.dram_tensor("wo1t", [E, E], FP32, kind="ExternalInput")
    wo2t_d = nc.dram_tensor("wo2t", [E, E], FP32, kind="ExternalInput")
    bq1_d = nc.dram_tensor("bq1", [E], FP32, kind="ExternalInput")
    bq2_d = nc.dram_tensor("bq2", [E], FP32, kind="ExternalInput")
    bk_d = nc.dram_tensor("bk", [E], FP32, kind="ExternalInput")
    bv_d = nc.dram_tensor("bv", [E], FP32, kind="ExternalInput")
    bo_d = nc.dram_tensor("bo", [E], FP32, kind="ExternalInput")
    lnw_d = nc.dram_tensor("lnw", [E], FP32, kind="ExternalInput")
    lnb_d = nc.dram_tensor("lnb", [E], FP32, kind="ExternalInput")
    out_d = nc.dram_tensor("out", [E, NQ], FP32, kind="ExternalOutput")

    with tile.TileContext(nc) as tc, ExitStack() as ctx:
        const = ctx.enter_context(tc.tile_pool(name="const", bufs=1))
        wts = ctx.enter_context(tc.tile_pool(name="wts", bufs=1))
        bigin = ctx.enter_context(tc.tile_pool(name="bigin", bufs=1))
        keep = ctx.enter_context(tc.tile_pool(name="keep", bufs=1))
        flow = ctx.enter_context(tc.tile_pool(name="flow", bufs=1))
        ps_s = ctx.enter_context(tc.tile_pool(name="ps_s", bufs=3, space="PSUM"))
        ps_o = ctx.enter_context(tc.tile_pool(name="ps_o", bufs=4, space="PSUM"))
        ps_d = ctx.enter_context(tc.tile_pool(name="ps_d", bufs=1, space="PSUM"))

        # ---- constants ----
        ident = const.tile([P, P], FP32, name="ident")
        make_identity(nc, ident)
        # fp32r matmuls need an even moving-operand innermost count, so the
        # denominator reduction uses a [P, 2] ones operand (N=2).
        ones_raw = const.tile([P, 2], FP32, name="ones_raw")
        nc.vector.memset(ones_raw, 1.0)
        ones = const.tile([P, 2], FP32R, name="ones")
        nc.vector.tensor_copy(ones[:], ones_raw[:])
        epst = const.tile([P, 1], FP32, name="epst")
        nc.vector.memset(epst, LN_EPS)

        # ---- weights / biases ----
        def _load_w(name, dram, ctiles):
            t = wts.tile([P, ctiles, E], FP32R, name=name)
            nc.sync.dma_start(
                t[:], dram[:].bitcast(FP32R).rearrange("(o p) e -> p o e", p=P)
            )
            return t

        wq1t = _load_w("wq1t", wq1t_d, CQT)
        wq2t = _load_w("wq2t", wq2t_d, CQT)
        wkt = _load_w("wkt", wkt_d, CT)
        wvt = _load_w("wvt", wvt_d, CT)
        wo1t = _load_w("wo1t", wo1t_d, ET)
        wo2t = _load_w("wo2t", wo2t_d, ET)

        bq1 = wts.tile([P, ET], FP32, name="bq1")
        nc.sync.dma_start(bq1[:], bq1_d[:].rearrange("(o p) -> p o", p=P))
        bq2 = wts.tile([P, ET], FP32, name="bq2")
        nc.sync.dma_start(bq2[:], bq2_d[:].rearrange("(o p) -> p o", p=P))
        bk = wts.tile([P, ET], FP32, name="bk")
        nc.sync.dma_start(bk[:], bk_d[:].rearrange("(o p) -> p o", p=P))

        bv_b = wts.tile([P, E], FP32, name="bv_b")
        _bcast_row(nc, bv_d, bv_b)
        bo_b = wts.tile([P, E], FP32, name="bo_b")
        _bcast_row(nc, bo_d, bo_b)
        lnw_b = wts.tile([P, E], FP32, name="lnw_b")
        _bcast_row(nc, lnw_d, lnw_b)
        lnb_b = wts.tile([P, E], FP32, name="lnb_b")
        _bcast_row(nc, lnb_d, lnb_b)

        # ---- phase 0: K^T, V, Q^T projections ----
        ktm = keep.tile([P, ET, N], FP32R, name="ktm")    # K^T e-major
        vtm = keep.tile([P, NKT, E], FP32R, name="vtm")   # V token-major

        KVCH = 512  # kv token-columns per streamed chunk
        for ch in range(N // KVCH):
            xkv_sb = bigin.tile([P, CT, KVCH], FP32R, name="xkv", tag="xkv", bufs=2)
            nc.sync.dma_start(
                xkv_sb[:],
                xkv_d[:].bitcast(FP32R).rearrange("(o p) n -> p o n", p=P)[
                    :, :, ch * KVCH : (ch + 1) * KVCH
                ],
            )
            # K^T for these token-columns
            for t in range(ET):
                for cc in range(KVCH // QB):
                    ps = ps_s.tile([P, QB], FP32, name="kps", tag="s")
                    for j in range(CT):
                        nc.tensor.matmul(
                            ps[:],
                            wkt[:, j, t * P : (t + 1) * P],
                            xkv_sb[:, j, cc * QB : (cc + 1) * QB],
                            start=(j == 0),
                            stop=(j == CT - 1),
                        )
                    nc.scalar.activation(
                        ktm[:, t, ch * KVCH + cc * QB : ch * KVCH + (cc + 1) * QB],
                        ps[:],
                        AF.Identity,
                        bias=bk[:, t : t + 1],
                        scale=1.0,
                    )
            # V for these token-rows
            for v in range(KVCH // P):
                kt_idx = ch * (KVCH // P) + v
                ps = ps_o.tile([P, E], FP32, name="vps", tag="o")
                for j in range(CT):
                    nc.tensor.matmul(
                        ps[:],
                        xkv_sb[:, j, v * P : (v + 1) * P],
                        wvt[:, j, :],
                        start=(j == 0),
                        stop=(j == CT - 1),
                    )
                nc.vector.tensor_tensor(vtm[:, kt_idx, :], ps[:], bv_b[:], ALU.add)

        qt1 = keep.tile([P, CQT, NQ], FP32R, name="qt1")  # Q1^T e-major
        qt2 = keep.tile([P, CQT, NQ], FP32R, name="qt2")
        QCH = 1024  # q token-columns per streamed chunk (double-buffered)
        for xq_d, wqt, bq, qt in ((xq1_d, wq1t, bq1, qt1), (xq2_d, wq2t, bq2, qt2)):
            for ch in range(NQ // QCH):
                csl = slice(ch * QCH, (ch + 1) * QCH)
                xq_sb = bigin.tile([P, CQT, QCH], FP32R, name="xq", tag="xq", bufs=2)
                nc.sync.dma_start(
                    xq_sb[:],
                    xq_d[:].bitcast(FP32R).rearrange("(o p) n -> p o n", p=P)[
                        :, :, csl
                    ],
                )
                for t in range(ET):
                    for cc in range(QCH // QB):
                        ps = ps_s.tile([P, QB], FP32, name="qps", tag="s")
                        for j in range(CQT):
                            nc.tensor.matmul(
                                ps[:],
                                wqt[:, j, t * P : (t + 1) * P],
                                xq_sb[:, j, cc * QB : (cc + 1) * QB],
                                start=(j == 0),
                                stop=(j == CQT - 1),
                            )
                        nc.scalar.activation(
                            qt[:, t, ch * QCH + cc * QB : ch * QCH + (cc + 1) * QB],
                            ps[:],
                            AF.Identity,
                            bias=bq[:, t : t + 1],
                            scale=1.0,
                        )

        # ---- phase 1: attention (both query sets) ----
        o1ut = keep.tile([P, ET, NQ], FP32R, name="o1ut")  # unnormalized out1^T
        o2ut = keep.tile([P, ET, NQ], FP32R, name="o2ut")
        r1 = keep.tile([P, NT], FP32, name="r1")           # 1/denom per token
        r2 = keep.tile([P, NT], FP32, name="r2")

        for qt, out_t, r_t in ((qt1, o1ut, r1), (qt2, o2ut, r2)):
            for qb in range(NQB):
                qsl = slice(qb * QB, (qb + 1) * QB)
                o_ps = [
                    ps_o.tile([P, QB], FP32, name=f"ops{t}", tag="o")
                    for t in range(ET)
                ]
                acc = flow.tile([P, QB], FP32R, name="acc", tag="acc", bufs=2)
                for k in range(NKT):
                    s_ps = ps_s.tile([P, QB], FP32, name="sps", tag="s")
                    for t in range(ET):
                        nc.tensor.matmul(
                            s_ps[:],
                            ktm[:, t, k * P : (k + 1) * P],
                            qt[:, t, qsl],
                            start=(t == 0),
                            stop=(t == ET - 1),
                        )
                    pt = flow.tile([P, QB], FP32R, name="pt", tag="pt", bufs=3)
                    nc.scalar.activation(pt[:], s_ps[:], AF.Exp, scale=SCALE)
                    for t in range(ET):
                        nc.tensor.matmul(
                            o_ps[t][:],
                            vtm[:, k, t * P : (t + 1) * P],
                            pt[:],
                            start=(k == 0),
                            stop=(k == NKT - 1),
                        )
                    if k == 0:
                        nc.vector.tensor_copy(acc[:], pt[:])
                    else:
                        nc.vector.tensor_tensor(acc[:], acc[:], pt[:], ALU.add)
                for t in range(ET):
                    nc.vector.tensor_copy(out_t[:, t, qsl], o_ps[t][:])
                # denominators -> [q, 1] via ones-vector matmuls (N=2 for
                # the fp32r even-element restriction; column 0 is used)
                d_ps = ps_d.tile([P, QB // P, 2], FP32, name="dps", tag="d")
                for i in range(QB // P):
                    nc.tensor.matmul(
                        d_ps[:, i, :],
                        acc[:, i * P : (i + 1) * P],
                        ones[:],
                        start=True,
                        stop=True,
                    )
                nc.vector.reciprocal(
                    r_t[:, qb * (QB // P) : (qb + 1) * (QB // P)], d_ps[:, :, 0]
                )

        # ---- phase 2a: out-proj + softmax-normalize + LayerNorm ----
        out_r = out_d[:].rearrange("(o p) n -> p o n", p=P)
        ynorms = []
        for nt in range(NT):
            nsl = slice(nt * P, (nt + 1) * P)
            y1_ps = ps_o.tile([P, E], FP32, name="y1ps", tag="o")
            for j in range(ET):
                nc.tensor.matmul(
                    y1_ps[:],
                    o1ut[:, j, nsl],
                    wo1t[:, j, :],
                    start=(j == 0),
                    stop=(j == ET - 1),
                )
            y2_ps = ps_o.tile([P, E], FP32, name="y2ps", tag="o")
            for j in range(ET):
                nc.tensor.matmul(
                    y2_ps[:],
                    o2ut[:, j, nsl],
                    wo2t[:, j, :],
                    start=(j == 0),
                    stop=(j == ET - 1),
                )
            # softmax 1/denom scaling on the scalar engine (per-partition
            # scale operand), combine + LN on vector
            y = flow.tile([P, E], FP32, name="y", tag="y", bufs=6)
            y2 = flow.tile([P, E], FP32, name="y2", tag="y2", bufs=3)
            nc.scalar.activation(
                y[:], y1_ps[:], AF.Identity, scale=r1[:, nt : nt + 1]
            )
            nc.scalar.activation(
                y2[:], y2_ps[:], AF.Identity, scale=r2[:, nt : nt + 1]
            )
            nc.vector.tensor_add(y[:], y[:], y2[:])
            nc.vector.tensor_add(y[:], y[:], bo_b[:])
            st6 = flow.tile([P, 6], FP32, name="st6", tag="st6", bufs=3)
            mv = flow.tile([P, 2], FP32, name="mv", tag="mv", bufs=3)
            nc.vector.bn_stats(out=st6[:], in_=y[:])
            nc.vector.bn_aggr(out=mv[:], in_=st6[:])
            rstd = flow.tile([P, 1], FP32, name="rstd", tag="rstd", bufs=3)
            nc.scalar.activation(
                rstd[:], mv[:, 1:2], AF.Sqrt, bias=epst[:], scale=1.0
            )
            nc.vector.reciprocal(rstd[:], rstd[:])
            nc.vector.tensor_scalar(
                y[:], y[:], mv[:, 0:1], rstd[:], op0=ALU.subtract, op1=ALU.mult
            )
            nc.vector.tensor_tensor(y[:], y[:], lnw_b[:], ALU.mult)
            nc.vector.tensor_tensor(y[:], y[:], lnb_b[:], ALU.add)
            ynorms.append(y)

        # ---- phase 2b: transpose to channel-major + store ----
        for nt in range(NT):
            nsl = slice(nt * P, (nt + 1) * P)
            y = ynorms[nt]
            yt = flow.tile([P, ET, P], FP32, name="yt", tag="yt", bufs=3)
            for t in range(ET):
                tp = ps_s.tile([P, P], FP32, name="tp", tag="s")
                nc.tensor.transpose(tp[:], y[:, t * P : (t + 1) * P], ident[:])
                nc.vector.tensor_copy(yt[:, t, :], tp[:])
            for t in range(ET):
                nc.sync.dma_start(out_r[:, t, nsl], yt[:, t, :])

    nc.compile()
    return nc


_CACHE = {}


def _get_nc():
    if "nc" not in _CACHE:
        _CACHE["nc"] = build_nc()
    return _CACHE["nc"]


def make_in_maps(q1, q2, kv, wq1, bq1, wq2, bq2, wk, bk, wv, bv, wo, bo, ln_w, ln_b):
    f32 = lambda a: np.ascontiguousarray(np.asarray(a, dtype=np.float32))
    q1, q2, kv = f32(q1), f32(q2), f32(kv)
    base = {
        "wq1t": f32(np.asarray(wq1).T),
        "wq2t": f32(np.asarray(wq2).T),
        "wkt": f32(np.asarray(wk).T),
        "wvt": f32(np.asarray(wv).T),
        "wo1t": f32(np.asarray(wo)[:, :E].T),
        "wo2t": f32(np.asarray(wo)[:, E:].T),
        "bq1": f32(bq1),
        "bq2": f32(bq2),
        "bk": f32(bk),
        "bv": f32(bv),
        "bo": f32(bo),
        "lnw": f32(ln_w),
        "lnb": f32(ln_b),
    }
    kv_flat = [f32(kv[b].reshape(CKV, N)) for b in range(B)]
    in_maps = []
    for c in range(8):
        b, h = divmod(c, 2)
        m = dict(base)
        m["xq1"] = f32(q1[b, :, h * 32 : (h + 1) * 32, :].reshape(CQ, NQ))
        m["xq2"] = f32(q2[b, :, h * 32 : (h + 1) * 32, :].reshape(CQ, NQ))
        m["xkv"] = kv_flat[b]
        in_maps.append(m)
    return in_maps


def assemble_output(results):
    out = np.empty((B, E, 64, 64), dtype=np.float32)
    for c in range(8):
        b, h = divmod(c, 2)
        out[b, :, h * 32 : (h + 1) * 32, :] = results[c]["out"].reshape(E, 32, 64)
    return out


def kernel(**inputs):
    from concourse.bass_utils import run_bass_kernel_spmd

    nc = _get_nc()
    in_maps = make_in_maps(**inputs)
    res = run_bass_kernel_spmd(nc, in_maps, list(range(8)))
    return assemble_output(res.results)


if __name__ == "__main__":
    nc = build_nc()
    print("built ok")


# revision 22
# speedup vs baseline: 1.2376x; 1.0215x over previous
"""Trainium2 Bass kernel for nn_CrossAttention_79448305041860.

Dual cross-attention (q1, q2 vs shared kv) + concat + out-proj + LayerNorm,
B=4, E=256, N=64*64=4096 tokens.

Sharding: 8 cores = 4 batches x 2 query-token halves. Each core computes
K,V for its batch (replicated across the pair of cores sharing a batch) and
the full pipeline for its 2048-query-token slice. No cross-core comm.

Per-core layout strategy:
  - K^T, Q^T computed E-major [e, tokens] straight from the channel-major
    inputs (contraction c on partitions) -- no input transposes.
  - Scores computed transposed: S^T[k, q] = (K^T-slice).T @ Q^T. exp() on
    the scalar engine (scale=1/16 folded in; scores are O(1) so no
    max-subtraction is needed for fp32 stability).
  - P^T tiles feed the PV matmul directly as the moving operand:
    out^T[e, q] += V[k-tile, e].T @ P^T[k-tile, q]  (V is token-major).
  - Softmax denominators: vector-engine accumulation of P^T tiles over k,
    then a tiny N=1 matmul with a ones-vector per 128 queries gives
    denom[q, 1] with q on partitions.
  - Out-proj consumes the *unnormalized* out^T halves as stationary
    operands: y_half[n-tile, e] = outT[f, n-tile].T @ woT[f, e]; the
    1/denom factors become native per-partition scalars there.
  - LayerNorm along the free dim (bn_stats/bn_aggr), then 128x128 PE
    transposes to the channel-major output layout.

All matmuls run in float32r (FP22 single-pass, full PE rate); walrus
requires fp32r matmul inputs to be produced as fp32r, so every matmul
input tile is allocated with that dtype.
"""

import numpy as np
from contextlib import ExitStack

import concourse.bass as bass
import concourse.mybir as mybir
import concourse.tile as tile
from concourse import bacc
from concourse.masks import make_identity

FP32 = mybir.dt.float32
FP32R = mybir.dt.float32r
AF = mybir.ActivationFunctionType
ALU = mybir.AluOpType

P = 128
B = 4
E = 256            # embed dim
ET = E // P        # 2 e-tiles
CKV = 512          # kv channels
CT = CKV // P      # 4 c-tiles
CQ = 256           # q channels
CQT = CQ // P      # 2 c-tiles
N = 4096           # kv tokens per batch
NKT = N // P       # 32 k token-tiles
NQ = 2048          # query tokens per core
QB = 512           # q block (psum bank width)
NQB = NQ // QB     # 4 q blocks
NT = NQ // P       # 16 token-tiles per core
SCALE = 1.0 / 16.0  # 1/sqrt(E)
LN_EPS = 1e-5


def _bcast_row(nc, dram_handle, sbuf_tile):
    """DMA-broadcast a [E] dram vector to all partitions of a [P, E] tile."""
    src_ap = dram_handle[:]
    bcast = bass.AP(
        tensor=src_ap.tensor,
        offset=src_ap.offset,
        ap=[[0, P], *src_ap.ap],
    )
    nc.gpsimd.dma_start(out=sbuf_tile[:], in_=bcast)


def build_nc():
    nc = bacc.Bacc()

    xq1_d = nc.dram_tensor("xq1", [CQ, NQ], FP32, kind="ExternalInput")
    xq2_d = nc.dram_tensor("xq2", [CQ, NQ], FP32, kind="ExternalInput")
    xkv_d = nc.dram_tensor("xkv", [CKV, N], FP32, kind="ExternalInput")
    wq1t_d = nc.dram_tensor("wq1t", [CQ, E], FP32, kind="ExternalInput")
    wq2t_d = nc.dram_tensor("wq2t", [CQ, E], FP32, kind="ExternalInput")
    wkt_d = nc.dram_tensor("wkt", [CKV, E], FP32, kind="ExternalInput")
    wvt_d = nc.dram_tensor("wvt", [CKV, E], FP32, kind="ExternalInput")
    wo1t_d = nc.dram_tensor("wo1t", [E, E], FP32, kind="ExternalInput")
    wo2t_d = nc.dram_tensor("wo2t", [E, E], FP32, kind="ExternalInput")
    bq1_d = nc.dram_tensor("bq1", [E], FP32, kind="ExternalInput")
    bq2_d = nc.dram_tensor("bq2", [E], FP32, kind="ExternalInput")
    bk_d = nc.dram_tensor("bk", [E], FP32, kind="ExternalInput")
    bv_d = nc.dram_tensor("bv", [E], FP32, kind="ExternalInput")
    bo_d = nc.dram_tensor("bo", [E], FP32, kind="ExternalInput")
    lnw_d = nc.dram_tensor("lnw", [E], FP32, kind="ExternalInput")
    lnb_d = nc.dram_tensor("lnb", [E], FP32, kind="ExternalInput")
    out_d = nc.dram_tensor("out", [E, NQ], FP32, kind="ExternalOutput")

    with tile.TileContext(nc) as tc, ExitStack() as ctx:
        const = ctx.enter_context(tc.tile_pool(name="const", bufs=1))
        wts = ctx.enter_context(tc.tile_pool(name="wts", bufs=1))
        bigin = ctx.enter_context(tc.tile_pool(name="bigin", bufs=1))
        keep = ctx.enter_context(tc.tile_pool(name="keep", bufs=1))
        flow = ctx.enter_context(tc.tile_pool(name="flow", bufs=1))
        ps_s = ctx.enter_context(tc.tile_pool(name="ps_s", bufs=3, space="PSUM"))
        ps_o = ctx.enter_context(tc.tile_pool(name="ps_o", bufs=4, space="PSUM"))
        ps_d = ctx.enter_context(tc.tile_pool(name="ps_d", bufs=1, space="PSUM"))

        # ---- constants ----
        ident = const.tile([P, P], FP32, name="ident")
        make_identity(nc, ident)
        # fp32r matmuls need an even moving-operand innermost count, so the
        # denominator reduction uses a [P, 2] ones operand (N=2).
        ones_raw = const.tile([P, 2], FP32, name="ones_raw")
        nc.vector.memset(ones_raw, 1.0)
        ones = const.tile([P, 2], FP32R, name="ones")
        nc.vector.tensor_copy(ones[:], ones_raw[:])
        epst = const.tile([P, 1], FP32, name="epst")
        nc.vector.memset(epst, LN_EPS)

        # ---- weights / biases ----
        def _load_w(name, dram, ctiles):
            t = wts.tile([P, ctiles, E], FP32R, name=name)
            nc.gpsimd.dma_start(
                t[:], dram[:].bitcast(FP32R).rearrange("(o p) e -> p o e", p=P)
            )
            return t

        wq1t = _load_w("wq1t", wq1t_d, CQT)
        wq2t = _load_w("wq2t", wq2t_d, CQT)
        wkt = _load_w("wkt", wkt_d, CT)
        wvt = _load_w("wvt", wvt_d, CT)
        wo1t = _load_w("wo1t", wo1t_d, ET)
        wo2t = _load_w("wo2t", wo2t_d, ET)

        bq1 = wts.tile([P, ET], FP32, name="bq1")
        nc.gpsimd.dma_start(bq1[:], bq1_d[:].rearrange("(o p) -> p o", p=P))
        bq2 = wts.tile([P, ET], FP32, name="bq2")
        nc.gpsimd.dma_start(bq2[:], bq2_d[:].rearrange("(o p) -> p o", p=P))
        bk = wts.tile([P, ET], FP32, name="bk")
        nc.gpsimd.dma_start(bk[:], bk_d[:].rearrange("(o p) -> p o", p=P))

        bv_b = wts.tile([P, E], FP32, name="bv_b")
        _bcast_row(nc, bv_d, bv_b)
        bo_b = wts.tile([P, E], FP32, name="bo_b")
        _bcast_row(nc, bo_d, bo_b)
        lnw_b = wts.tile([P, E], FP32, name="lnw_b")
        _bcast_row(nc, lnw_d, lnw_b)
        lnb_b = wts.tile([P, E], FP32, name="lnb_b")
        _bcast_row(nc, lnb_d, lnb_b)

        # ---- phase 0: K^T, V, Q^T projections ----
        ktm = keep.tile([P, ET, N], FP32R, name="ktm")    # K^T e-major
        vtm = keep.tile([P, NKT, E], FP32R, name="vtm")   # V token-major

        KVCH = 512  # kv token-columns per streamed chunk
        for ch in range(N // KVCH):
            xkv_sb = bigin.tile([P, CT, KVCH], FP32R, name="xkv", tag="xkv", bufs=3)
            nc.sync.dma_start(
                xkv_sb[:],
                xkv_d[:].bitcast(FP32R).rearrange("(o p) n -> p o n", p=P)[
                    :, :, ch * KVCH : (ch + 1) * KVCH
                ],
            )
            # K^T for these token-columns
            for t in range(ET):
                for cc in range(KVCH // QB):
                    ps = ps_s.tile([P, QB], FP32, name="kps", tag="s")
                    for j in range(CT):
                        nc.tensor.matmul(
                            ps[:],
                            wkt[:, j, t * P : (t + 1) * P],
                            xkv_sb[:, j, cc * QB : (cc + 1) * QB],
                            start=(j == 0),
                            stop=(j == CT - 1),
                        )
                    nc.scalar.activation(
                        ktm[:, t, ch * KVCH + cc * QB : ch * KVCH + (cc + 1) * QB],
                        ps[:],
                        AF.Identity,
                        bias=bk[:, t : t + 1],
                        scale=1.0,
                    )
            # V for these token-rows
            for v in range(KVCH // P):
                kt_idx = ch * (KVCH // P) + v
                ps = ps_o.tile([P, E], FP32, name="vps", tag="o")
                for j in range(CT):
                    nc.tensor.matmul(
                        ps[:],
                        xkv_sb[:, j, v * P : (v + 1) * P],
                        wvt[:, j, :],
                        start=(j == 0),
                        stop=(j == CT - 1),
                    )
                nc.vector.tensor_tensor(vtm[:, kt_idx, :], ps[:], bv_b[:], ALU.add)

        qt1 = keep.tile([P, CQT, NQ], FP32R, name="qt1")  # Q1^T e-major
        qt2 = keep.tile([P, CQT, NQ], FP32R, name="qt2")
        QCH = 512  # q token-columns per streamed chunk (double-buffered)
        for xq_d, wqt, bq, qt in ((xq1_d, wq1t, bq1, qt1), (xq2_d, wq2t, bq2, qt2)):
            for ch in range(NQ // QCH):
                csl = slice(ch * QCH, (ch + 1) * QCH)
                xq_sb = bigin.tile([P, CQT, QCH], FP32R, name="xq", tag="xq", bufs=2)
                nc.sync.dma_start(
                    xq_sb[:],
                    xq_d[:].bitcast(FP32R).rearrange("(o p) n -> p o n", p=P)[
                        :, :, csl
                    ],
                )
                for t in range(ET):
                    for cc in range(QCH // QB):
                        ps = ps_s.tile([P, QB], FP32, name="qps", tag="s")
                        for j in range(CQT):
                            nc.tensor.matmul(
                                ps[:],
                                wqt[:, j, t * P : (t + 1) * P],
                                xq_sb[:, j, cc * QB : (cc + 1) * QB],
                                start=(j == 0),
                                stop=(j == CQT - 1),
                            )
                        nc.scalar.activation(
                            qt[:, t, ch * QCH + cc * QB : ch * QCH + (cc + 1) * QB],
                            ps[:],
                            AF.Identity,
                            bias=bq[:, t : t + 1],
                            scale=1.0,
                        )

        # ---- phase 1: attention (both query sets) ----
        o1ut = keep.tile([P, ET, NQ], FP32R, name="o1ut")  # unnormalized out1^T
        o2ut = keep.tile([P, ET, NQ], FP32R, name="o2ut")
        r1 = keep.tile([P, NT], FP32, name="r1")           # 1/denom per token
        r2 = keep.tile([P, NT], FP32, name="r2")

        for qt, out_t, r_t in ((qt1, o1ut, r1), (qt2, o2ut, r2)):
            for qb in range(NQB):
                qsl = slice(qb * QB, (qb + 1) * QB)
                o_ps = [
                    ps_o.tile([P, QB], FP32, name=f"ops{t}", tag="o")
                    for t in range(ET)
                ]
                acc = flow.tile([P, QB], FP32R, name="acc", tag="acc", bufs=2)
                for k in range(NKT):
                    s_ps = ps_s.tile([P, QB], FP32, name="sps", tag="s")
                    for t in range(ET):
                        nc.tensor.matmul(
                            s_ps[:],
                            ktm[:, t, k * P : (k + 1) * P],
                            qt[:, t, qsl],
                            start=(t == 0),
                            stop=(t == ET - 1),
                        )
                    pt = flow.tile([P, QB], FP32R, name="pt", tag="pt", bufs=3)
                    nc.scalar.activation(pt[:], s_ps[:], AF.Exp, scale=SCALE)
                    for t in range(ET):
                        nc.tensor.matmul(
                            o_ps[t][:],
                            vtm[:, k, t * P : (t + 1) * P],
                            pt[:],
                            start=(k == 0),
                            stop=(k == NKT - 1),
                        )
                    if k == 0:
                        nc.vector.tensor_copy(acc[:], pt[:])
                    else:
                        nc.vector.tensor_tensor(acc[:], acc[:], pt[:], ALU.add)
                for t in range(ET):
                    nc.vector.tensor_copy(out_t[:, t, qsl], o_ps[t][:])
                # denominators -> [q, 1] via ones-vector matmuls (N=2 for
                # the fp32r even-element restriction; column 0 is used)
                d_ps = ps_d.tile([P, QB // P, 2], FP32, name="dps", tag="d")
                for i in range(QB // P):
                    nc.tensor.matmul(
                        d_ps[:, i, :],
                        acc[:, i * P : (i + 1) * P],
                        ones[:],
                        start=True,
                        stop=True,
                    )
                nc.vector.reciprocal(
                    r_t[:, qb * (QB // P) : (qb + 1) * (QB // P)], d_ps[:, :, 0]
                )

        # ---- phase 2a: out-proj + softmax-normalize + LayerNorm ----
        # Token tiles processed in pairs: the per-channel elementwise ops run
        # on [P, 2, E] shapes (halves DVE op count); per-token stats stay
        # per-tile. The +bo / +lnb adds ride on the otherwise-idle gpsimd.
        out_r = out_d[:].rearrange("(o p) n -> p o n", p=P)
        bo_bc = bo_b[:, None, :].to_broadcast([P, 2, E])
        lnw_bc = lnw_b[:, None, :].to_broadcast([P, 2, E])
        lnb_bc = lnb_b[:, None, :].to_broadcast([P, 2, E])
        ynorms = []
        for pr in range(NT // 2):
            y = flow.tile([P, 2, E], FP32, name="y", tag="y", bufs=4)
            y2 = flow.tile([P, 2, E], FP32, name="y2", tag="y2", bufs=2)
            for h in range(2):
                nt = pr * 2 + h
                nsl = slice(nt * P, (nt + 1) * P)
                y1_ps = ps_o.tile([P, E], FP32, name="y1ps", tag="o")
                for j in range(ET):
                    nc.tensor.matmul(
                        y1_ps[:],
                        o1ut[:, j, nsl],
                        wo1t[:, j, :],
                        start=(j == 0),
                        stop=(j == ET - 1),
                    )
                y2_ps = ps_o.tile([P, E], FP32, name="y2ps", tag="o")
                for j in range(ET):
                    nc.tensor.matmul(
                        y2_ps[:],
                        o2ut[:, j, nsl],
                        wo2t[:, j, :],
                        start=(j == 0),
                        stop=(j == ET - 1),
                    )
                # softmax 1/denom scaling on the scalar engine
                nc.scalar.activation(
                    y[:, h, :], y1_ps[:], AF.Identity, scale=r1[:, nt : nt + 1]
                )
                nc.scalar.activation(
                    y2[:, h, :], y2_ps[:], AF.Identity, scale=r2[:, nt : nt + 1]
                )
            nc.vector.tensor_tensor(y[:], y[:], y2[:], ALU.add)
            nc.gpsimd.tensor_tensor(y[:], y[:], bo_bc, ALU.add)
            mvs = []
            for h in range(2):
                st6 = flow.tile([P, 6], FP32, name="st6", tag="st6", bufs=3)
                mv = flow.tile([P, 2], FP32, name="mv", tag="mv", bufs=4)
                nc.vector.bn_stats(out=st6[:], in_=y[:, h, :])
                nc.vector.bn_aggr(out=mv[:], in_=st6[:])
                rstd = flow.tile([P, 1], FP32, name="rstd", tag="rstd", bufs=4)
                nc.scalar.activation(
                    rstd[:], mv[:, 1:2], AF.Sqrt, bias=epst[:], scale=1.0
                )
                nc.vector.reciprocal(rstd[:], rstd[:])
                nc.vector.tensor_scalar(
                    y[:, h, :],
                    y[:, h, :],
                    mv[:, 0:1],
                    rstd[:],
                    op0=ALU.subtract,
                    op1=ALU.mult,
                )
            nc.vector.tensor_tensor(y[:], y[:], lnw_bc, ALU.mult)
            nc.gpsimd.tensor_tensor(y[:], y[:], lnb_bc, ALU.add)
            ynorms.append(y)

        # ---- phase 2b: transpose to channel-major + store ----
        for nt in range(NT):
            nsl = slice(nt * P, (nt + 1) * P)
            y = ynorms[nt // 2][:, nt % 2, :]
            yt = flow.tile([P, ET, P], FP32, name="yt", tag="yt", bufs=3)
            for t in range(ET):
                tp = ps_s.tile([P, P], FP32, name="tp", tag="s")
                nc.tensor.transpose(tp[:], y[:, t * P : (t + 1) * P], ident[:])
                nc.vector.tensor_copy(yt[:, t, :], tp[:])
            for t in range(ET):
                nc.sync.dma_start(out_r[:, t, nsl], yt[:, t, :])

    nc.compile()
    return nc


_CACHE = {}


def _get_nc():
    if "nc" not in _CACHE:
        _CACHE["nc"] = build_nc()
    return _CACHE["nc"]


def make_in_maps(q1, q2, kv, wq1, bq1, wq2, bq2, wk, bk, wv, bv, wo, bo, ln_w, ln_b):
    f32 = lambda a: np.ascontiguousarray(np.asarray(a, dtype=np.float32))
    q1, q2, kv = f32(q1), f32(q2), f32(kv)
    base = {
        "wq1t": f32(np.asarray(wq1).T),
        "wq2t": f32(np.asarray(wq2).T),
        "wkt": f32(np.asarray(wk).T),
        "wvt": f32(np.asarray(wv).T),
        "wo1t": f32(np.asarray(wo)[:, :E].T),
        "wo2t": f32(np.asarray(wo)[:, E:].T),
        "bq1": f32(bq1),
        "bq2": f32(bq2),
        "bk": f32(bk),
        "bv": f32(bv),
        "bo": f32(bo),
        "lnw": f32(ln_w),
        "lnb": f32(ln_b),
    }
    kv_flat = [f32(kv[b].reshape(CKV, N)) for b in range(B)]
    in_maps = []
    for c in range(8):
        b, h = divmod(c, 2)
        m = dict(base)
        m["xq1"] = f32(q1[b, :, h * 32 : (h + 1) * 32, :].reshape(CQ, NQ))
        m["xq2"] = f32(q2[b, :, h * 32 : (h + 1) * 32, :].reshape(CQ, NQ))
        m["xkv"] = kv_flat[b]
        in_maps.append(m)
    return in_maps


def assemble_output(results):
    out = np.empty((B, E, 64, 64), dtype=np.float32)
    for c in range(8):
        b, h = divmod(c, 2)
        out[b, :, h * 32 : (h + 1) * 32, :] = results[c]["out"].reshape(E, 32, 64)
    return out


def kernel(**inputs):
    from concourse.bass_utils import run_bass_kernel_spmd

    nc = _get_nc()
    in_maps = make_in_maps(**inputs)
    res = run_bass_kernel_spmd(nc, in_maps, list(range(8)))
    return assemble_output(res.results)


if __name__ == "__main__":
    nc = build_nc()
    print("built ok")
